# revision 13
# baseline (speedup 1.0000x reference)
"""Multi-head attention kernel for 8 Trainium2 NeuronCores (v3).

Problem: B=4, L=2048, DIM=1024, H=16 heads, d_k=d_v=64.
Sharding: data-parallel over (batch, query-half); full K/V replicated
to both cores of a batch pair at input staging (untimed) -> zero
collectives; each core projects all 2048 kv tokens itself.

Per-core dataflow (bf16 matmuls, fp32 PSUM):
  prologue: V-proj (vhp[st] [128s,16h,64], data-stationary), K(0), Q(0)
  per head-pair hp (blocks):
    per key tile kt:
      scores: 4 matmuls into ONE [128,2048] PSUM tile
        [A|B] x [q0|q1]; A = PE rows 0:64 (tile_position (0,0)),
        B = rows 64:128 ((64,0)) - concurrent row-tiled K=64 pairs.
      ONE 2us ACT(exp, scale=1/32) over all 2048 -> exp tile bf16.
      PV: op_q[128,512] rows 0:64 += V_A.T @ exp_A (tile (0,0)),
          rows 64:128 += V_B.T @ exp_B ((0,64)) - col-concurrent.
      den: 4 concurrent M=1 matmuls (ones stationary) -> den bank
          rows 0/32/64/96 = A_q0/B_q0/A_q1/B_q1.
      weave slot: K/Q projection chunks of hp+1 fill the ACT-gated
          tensor idle time (engine queues are FIFO in emission order,
          so overlap must be emitted interleaved).
    norm: 4x recip -> DRAM bounce -> partition-bcast into bc[0:64]/
          [64:128] -> 2 full [128,512] muls into oall[hp] (PACKED:
          head A rows 0:64, head B rows 64:128; no partition shift
          needed since PV already placed B at 64:128).
  C: yT[dt] = sum_hp pwsb[hp][:,dt-chunk].T @ oall[hp] + bias
     (8 full-K=128 chunks).

PSUM map (8 banks): sc [128,2048] (4) + op bufs=2 (2) + den (1) +
pp [128,512] for woven projections (1).
"""

import numpy as np

P = 128
B, L, DIM, H, DK = 4, 2048, 1024, 16, 64
TQ = 1024      # q tokens per core
TS = 2048      # kv tokens per core (full batch)
NDCH = DIM // P          # 8 contraction chunks
NHP = H // 2             # 8 head pairs
NST = TS // P            # 16 key tiles
N_CORES = 8

_NC = None
TRACE = False
LAST_RESULT = None


def _build():
    import concourse.bass as bass
    from concourse import bacc
    import concourse.mybir as mybir
    import concourse.tile as tile

    DT_B = mybir.dt.bfloat16
    DT_F = mybir.dt.float32
    AF = mybir.ActivationFunctionType

    nc = bacc.Bacc(None, target_bir_lowering=False)
    qT = nc.dram_tensor("qT", [DIM, TQ], DT_B, kind="ExternalInput")
    kT = nc.dram_tensor("kT", [DIM, TS], DT_B, kind="ExternalInput")
    vT = nc.dram_tensor("vT", [DIM, TS], DT_B, kind="ExternalInput")
    wq = nc.dram_tensor("wq", [DIM, H * DK], DT_B, kind="ExternalInput")
    wk = nc.dram_tensor("wk", [DIM, H * DK], DT_B, kind="ExternalInput")
    wv = nc.dram_tensor("wv", [DIM, H * DK], DT_B, kind="ExternalInput")
    pw = nc.dram_tensor("pwT", [H * DK, DIM], DT_B, kind="ExternalInput")
    pb = nc.dram_tensor("pb", [P, NDCH], DT_F, kind="ExternalInput")
    yT = nc.dram_tensor("yT", [DIM, TQ], DT_F, kind="ExternalOutput")

    def bcast_ap(ap, count):
        return bass.AP(tensor=ap.tensor, offset=ap.offset,
                       ap=[[0, count]] + [list(x) for x in ap.ap[1:]])

    with tile.TileContext(nc) as tc, \
         tc.tile_pool(name="l1", bufs=1) as l1, \
         tc.tile_pool(name="exp_pool", bufs=5) as expp, \
         tc.tile_pool(name="scpsum", bufs=1, space="PSUM") as scps, \
         tc.tile_pool(name="opsum", bufs=2, space="PSUM") as ops, \
         tc.tile_pool(name="denpsum", bufs=1, space="PSUM") as dps, \
         tc.tile_pool(name="ppsum", bufs=1, space="PSUM") as pps, \
         tc.tile_pool(name="sums_pool", bufs=2) as smp, \
         tc.tile_pool(name="bc_pool", bufs=2) as bcp, \
         tc.tile_pool(name="bounce", bufs=4, space="DRAM") as bncp:

        # ---- whole-program tiles ----
        kht = [l1.tile([P, TS], DT_B, name=f"kht{i}") for i in range(NHP)]
        qht = [l1.tile([P, TQ], DT_B, name=f"qht{i}") for i in range(NHP)]
        vhp = [l1.tile([P, H, DK], DT_B, name=f"vhp{i}")
               for i in range(NST)]
        oall = [l1.tile([P, TQ], DT_B, name=f"oall{i}") for i in range(NHP)]
        pbt = l1.tile([P, NDCH], DT_F, name="pbt")
        ones = l1.tile([P, 1], DT_B, name="ones")
        nc.sync.dma_start(out=pbt[:, :], in_=pb[:, :])
        nc.vector.memset(ones[:, :], 1.0)

        # ---------- emit helpers ----------
        def emit_kproj_q(hp, j, kin, wkt):
            # quarter j of kht[hp]: token cols j*512:(j+1)*512
            ps = pps.tile([P, 512], DT_F, name=f"kps_{hp}_{j}", tag="pp")
            for d in range(NDCH):
                nc.tensor.matmul(
                    ps[:, :],
                    wkt[d][:, hp * P:(hp + 1) * P],
                    kin[d][:, j * 512:(j + 1) * 512],
                    start=(d == 0), stop=(d == NDCH - 1))
            nc.vector.tensor_copy(kht[hp][:, j * 512:(j + 1) * 512],
                                  ps[:, :])

        def emit_qproj_h(hp, m, qin, wqt):
            ps = pps.tile([P, 512], DT_F, name=f"qps_{hp}_{m}", tag="pp")
            for d in range(NDCH):
                nc.tensor.matmul(
                    ps[:, :],
                    wqt[d][:, hp * P:(hp + 1) * P],
                    qin[d][:, m * 512:(m + 1) * 512],
                    start=(d == 0), stop=(d == NDCH - 1))
            nc.vector.tensor_copy(qht[hp][:, m * 512:(m + 1) * 512],
                                  ps[:, :])

        def emit_vproj(st, vin, wvt):
            # vhp[st][s, h, dk] for all 16 heads; sc-tag psum (prologue)
            ps = scps.tile([P, 2048], DT_F, name=f"vps_{st}", tag="sc")
            for d in range(NDCH):
                for m in range(2):
                    nc.tensor.matmul(
                        ps[:, m * 512:(m + 1) * 512],
                        vin[d][:, st * P:(st + 1) * P],
                        wvt[d][:, m * 512:(m + 1) * 512],
                        start=(d == 0), stop=(d == NDCH - 1))
            nc.vector.tensor_copy(
                vhp[st][:, :, :],
                ps[:, 0:1024].rearrange("p (h d) -> p h d", d=DK))

        def emit_block(hp, weave):
            hA, hB = 2 * hp, 2 * hp + 1
            opq = [ops.tile([P, 512], DT_F, name=f"op_{hp}_{q}", tag="op")
                   for q in range(2)]
            den = dps.tile([P, 512], DT_F, name=f"den_{hp}", tag="den")
            for kt in range(NST):
                sc = scps.tile([P, 2048], DT_F, name=f"sc_{hp}_{kt}",
                               tag="sc")
                for q in range(2):
                    nc.tensor.matmul(
                        sc[:, (2 * q) * 512:(2 * q + 1) * 512],
                        kht[hp][0:DK, kt * P:(kt + 1) * P],
                        qht[hp][0:DK, q * 512:(q + 1) * 512],
                        start=True, stop=True)
                    nc.tensor.matmul(
                        sc[:, (2 * q + 1) * 512:(2 * q + 2) * 512],
                        kht[hp][DK:P, kt * P:(kt + 1) * P],
                        qht[hp][DK:P, q * 512:(q + 1) * 512],
                        start=True, stop=True)
                ex = expp.tile([P, 2048], DT_B, name=f"exp_{hp}_{kt}",
                               tag="exp")
                nc.scalar.activation(ex[:, :], sc[:, :], AF.Exp,
                                     scale=1.0 / 32.0)
                # PV: [A|B] col-concurrent per q-half
                for q in range(2):
                    nc.tensor.matmul(
                        opq[q][0:DK, :], vhp[kt][:, hA, :],
                        ex[:, (2 * q) * 512:(2 * q + 1) * 512],
                        start=(kt == 0), stop=(kt == NST - 1))
                    nc.tensor.matmul(
                        opq[q][DK:P, :], vhp[kt][:, hB, :],
                        ex[:, (2 * q + 1) * 512:(2 * q + 2) * 512],
                        start=(kt == 0), stop=(kt == NST - 1))
                # denominators: 4 concurrent M=1 col-tiled matmuls
                for j in range(4):
                    nc.tensor.matmul(
                        den[32 * j:32 * j + 1, :], ones[:, :],
                        ex[:, j * 512:(j + 1) * 512],
                        start=(kt == 0), stop=(kt == NST - 1),
                        tile_position=(0, 32 * j))
                if kt < len(weave):
                    weave[kt]()
            # normalization
            sm = smp.tile([P, 512], DT_F, name=f"sm_{hp}", tag="sm")
            for j in range(4):
                nc.vector.reciprocal(sm[32 * j:32 * j + 1, :],
                                     den[32 * j:32 * j + 1, :])
            bcq = [bcp.tile([P, 512], DT_F, name=f"bc_{hp}_{q}", tag="bc")
                   for q in range(2)]
            # den rows 32*j: j=0 A_q0, 1 B_q0, 2 A_q1, 3 B_q1
            for j in range(4):
                q, side = j // 2, j % 2
                bn = bncp.tile([1, 512], DT_F, name=f"bn_{hp}_{j}",
                               tag="bn")
                nc.sync.dma_start(out=bn[:, :],
                                  in_=sm[32 * j:32 * j + 1, :])
                nc.sync.dma_start(
                    out=bcq[q][side * DK:(side + 1) * DK, :],
                    in_=bcast_ap(bn[0:1, :], DK))
            for q in range(2):
                nc.vector.tensor_mul(oall[hp][:, q * 512:(q + 1) * 512],
                                     opq[q][:, :], bcq[q][:, :])

        # ---------- program ----------
        # prologue: V-proj all st, K(0), Q(0)
        with tc.tile_pool(name="v_in", bufs=1) as vip:
            vin = [vip.tile([P, TS], DT_B, name=f"vin{d}")
                   for d in range(NDCH)]
            wvt = [vip.tile([P, H * DK], DT_B, name=f"wvt{d}")
                   for d in range(NDCH)]
            for d in range(NDCH):
                nc.sync.dma_start(out=vin[d][:, :],
                                  in_=vT[d * P:(d + 1) * P, :])
                nc.scalar.dma_start(out=wvt[d][:, :],
                                    in_=wv[d * P:(d + 1) * P, :])
            for st in range(NST):
                emit_vproj(st, vin, wvt)

        with tc.tile_pool(name="kq_in", bufs=1) as kqp:
            kin = [kqp.tile([P, TS], DT_B, name=f"kin{d}")
                   for d in range(NDCH)]
            wkt = [kqp.tile([P, H * DK], DT_B, name=f"wkt{d}")
                   for d in range(NDCH)]
            qin = [kqp.tile([P, TQ], DT_B, name=f"qin{d}")
                   for d in range(NDCH)]
            wqt = [kqp.tile([P, H * DK], DT_B, name=f"wqt{d}")
                   for d in range(NDCH)]
            for d in range(NDCH):
                nc.sync.dma_start(out=kin[d][:, :],
                                  in_=kT[d * P:(d + 1) * P, :])
                nc.scalar.dma_start(out=wkt[d][:, :],
                                    in_=wk[d * P:(d + 1) * P, :])
                nc.gpsimd.dma_start(out=qin[d][:, :],
                                    in_=qT[d * P:(d + 1) * P, :])
                nc.gpsimd.dma_start(out=wqt[d][:, :],
                                    in_=wq[d * P:(d + 1) * P, :])

            for j in range(4):
                emit_kproj_q(0, j, kin, wkt)
            for m in range(2):
                emit_qproj_h(0, m, qin, wqt)

            def weave_for(hp):
                # projection chunks for head pair hp, spread over slots
                if hp >= NHP:
                    return []
                w = []
                for j in range(4):
                    w.append(lambda j=j: emit_kproj_q(hp, j, kin, wkt))
                for m in range(2):
                    w.append(lambda m=m: emit_qproj_h(hp, m, qin, wqt))
                return w

            for hp in range(NHP):
                # spread the ~6 weave thunks across the 16 kt slots
                thunks = weave_for(hp + 1)
                slots = [None] * NST
                for i, t in enumerate(thunks):
                    slots[2 + 2 * i] = t
                weave = [t if t is not None else (lambda: None)
                         for t in slots]
                emit_block(hp, weave)

        # ---- phase C ----
        with tc.tile_pool(name="pw_pool", bufs=1) as pwp, \
             tc.tile_pool(name="yst_pool", bufs=2) as ystp:
            pwsb = [pwp.tile([P, DIM], DT_B, name=f"pwsb{i}")
                    for i in range(NHP)]
            for hp in range(NHP):
                nc.gpsimd.dma_start(out=pwsb[hp][:, :],
                                    in_=pw[hp * P:(hp + 1) * P, :])
            for dt_ in range(NDCH):
                ps = scps.tile([P, 2048], DT_F, name=f"yps_{dt_}",
                               tag="sc")
                for hp in range(NHP):
                    for m in range(2):
                        nc.tensor.matmul(
                            ps[:, m * 512:(m + 1) * 512],
                            pwsb[hp][:, dt_ * P:(dt_ + 1) * P],
                            oall[hp][:, m * 512:(m + 1) * 512],
                            start=(hp == 0), stop=(hp == NHP - 1))
                yst = ystp.tile([P, TQ], DT_F, name=f"yst_{dt_}",
                                tag="yst")
                nc.vector.tensor_scalar_add(yst[:, :], ps[:, 0:1024],
                                            pbt[:, dt_:dt_ + 1])
                nc.sync.dma_start(
                    out=yT[dt_ * P:(dt_ + 1) * P, :], in_=yst[:, :])

    nc.compile()
    return nc


def kernel(q, k, v, w_q, w_k, w_v, proj_w, proj_b):
    global _NC, LAST_RESULT
    import ml_dtypes
    from concourse.bass_utils import run_bass_kernel_spmd

    if _NC is None:
        _NC = _build()

    bf16 = ml_dtypes.bfloat16
    q = np.asarray(q, dtype=np.float32)
    k = np.asarray(k, dtype=np.float32)
    v = np.asarray(v, dtype=np.float32)
    w_q = np.asarray(w_q, dtype=np.float32)
    w_k = np.asarray(w_k, dtype=np.float32)
    w_v = np.asarray(w_v, dtype=np.float32)
    proj_w = np.asarray(proj_w, dtype=np.float32)
    proj_b = np.asarray(proj_b, dtype=np.float32)

    wq2 = np.ascontiguousarray(
        np.transpose(w_q, (1, 0, 2)).reshape(DIM, H * DK)).astype(bf16)
    wk2 = np.ascontiguousarray(
        np.transpose(w_k, (1, 0, 2)).reshape(DIM, H * DK)).astype(bf16)
    wv2 = np.ascontiguousarray(
        np.transpose(w_v, (1, 0, 2)).reshape(DIM, H * DK)).astype(bf16)
    pwT = np.ascontiguousarray(proj_w.T).astype(bf16)
    pb2 = np.ascontiguousarray(proj_b.reshape(NDCH, P).T)

    in_maps = []
    for c in range(N_CORES):
        b, qo = c // 2, c % 2
        if qo == 0:
            kTb = np.ascontiguousarray(k[b].T).astype(bf16)
            vTb = np.ascontiguousarray(v[b].T).astype(bf16)
        in_maps.append({
            "qT": np.ascontiguousarray(
                q[b, qo * TQ:(qo + 1) * TQ, :].T).astype(bf16),
            "kT": kTb,
            "vT": vTb,
            "wq": wq2, "wk": wk2, "wv": wv2,
            "pwT": pwT, "pb": pb2,
        })

    res = run_bass_kernel_spmd(_NC, in_maps, list(range(N_CORES)), trace=TRACE)
    LAST_RESULT = res

    out = np.empty((B, L, DIM), dtype=np.float32)
    for c in range(N_CORES):
        b, qo = c // 2, c % 2
        out[b, qo * TQ:(qo + 1) * TQ, :] = res.results[c]["yT"].T
    return out


# revision 18
# speedup vs baseline: 1.4370x; 1.4370x over previous
"""Multi-head attention kernel for 8 Trainium2 NeuronCores (v3).

Problem: B=4, L=2048, DIM=1024, H=16 heads, d_k=d_v=64.
Sharding: data-parallel over (batch, query-half); full K/V replicated
to both cores of a batch pair at input staging (untimed) -> zero
collectives; each core projects all 2048 kv tokens itself.

Per-core dataflow (bf16 matmuls, fp32 PSUM):
  prologue: V-proj (vhp[st] [128s,16h,64], data-stationary), K(0), Q(0)
  per head-pair hp (blocks):
    per key tile kt:
      scores: 4 matmuls into ONE [128,2048] PSUM tile
        [A|B] x [q0|q1]; A = PE rows 0:64 (tile_position (0,0)),
        B = rows 64:128 ((64,0)) - concurrent row-tiled K=64 pairs.
      ONE 2us ACT(exp, scale=1/32) over all 2048 -> exp tile bf16.
      PV: op_q[128,512] rows 0:64 += V_A.T @ exp_A (tile (0,0)),
          rows 64:128 += V_B.T @ exp_B ((0,64)) - col-concurrent.
      den: 4 concurrent M=1 matmuls (ones stationary) -> den bank
          rows 0/32/64/96 = A_q0/B_q0/A_q1/B_q1.
      weave slot: K/Q projection chunks of hp+1 fill the ACT-gated
          tensor idle time (engine queues are FIFO in emission order,
          so overlap must be emitted interleaved).
    norm: 4x recip -> DRAM bounce -> partition-bcast into bc[0:64]/
          [64:128] -> 2 full [128,512] muls into oall[hp] (PACKED:
          head A rows 0:64, head B rows 64:128; no partition shift
          needed since PV already placed B at 64:128).
  C: yT[dt] = sum_hp pwsb[hp][:,dt-chunk].T @ oall[hp] + bias
     (8 full-K=128 chunks).

PSUM map (8 banks): sc [128,2048] (4) + op bufs=2 (2) + den (1) +
pp [128,512] for woven projections (1).
"""

import numpy as np

P = 128
B, L, DIM, H, DK = 4, 2048, 1024, 16, 64
TQ = 1024      # q tokens per core
TS = 2048      # kv tokens per core (full batch)
NDCH = DIM // P          # 8 contraction chunks
NHP = H // 2             # 8 head pairs
NST = TS // P            # 16 key tiles
N_CORES = 8

_NC = None
TRACE = False
LAST_RESULT = None


def _build():
    import concourse.bass as bass
    from concourse import bacc
    import concourse.mybir as mybir
    import concourse.tile as tile

    DT_B = mybir.dt.bfloat16
    DT_F = mybir.dt.float32
    AF = mybir.ActivationFunctionType

    nc = bacc.Bacc(None, target_bir_lowering=False)
    qT = nc.dram_tensor("qT", [DIM, TQ], DT_B, kind="ExternalInput")
    kT = nc.dram_tensor("kT", [DIM, TS], DT_B, kind="ExternalInput")
    vT = nc.dram_tensor("vT", [DIM, TS], DT_B, kind="ExternalInput")
    wq = nc.dram_tensor("wq", [DIM, H * DK], DT_B, kind="ExternalInput")
    wk = nc.dram_tensor("wk", [DIM, H * DK], DT_B, kind="ExternalInput")
    wv = nc.dram_tensor("wv", [DIM, H * DK], DT_B, kind="ExternalInput")
    pw = nc.dram_tensor("pwT", [H * DK, DIM], DT_B, kind="ExternalInput")
    pb = nc.dram_tensor("pb", [P, NDCH], DT_F, kind="ExternalInput")
    yT = nc.dram_tensor("yT", [DIM, TQ], DT_F, kind="ExternalOutput")

    def bcast_ap(ap, count):
        return bass.AP(tensor=ap.tensor, offset=ap.offset,
                       ap=[[0, count]] + [list(x) for x in ap.ap[1:]])

    with tile.TileContext(nc) as tc, \
         tc.tile_pool(name="l1", bufs=1) as l1, \
         tc.tile_pool(name="exp_pool", bufs=10) as expp, \
         tc.tile_pool(name="scpsum", bufs=2, space="PSUM") as scps, \
         tc.tile_pool(name="opsum", bufs=2, space="PSUM") as ops, \
         tc.tile_pool(name="denpsum", bufs=1, space="PSUM") as dps, \
         tc.tile_pool(name="ppsum", bufs=1, space="PSUM") as pps, \
         tc.tile_pool(name="sums_pool", bufs=2) as smp, \
         tc.tile_pool(name="bc_pool", bufs=2) as bcp, \
         tc.tile_pool(name="bounce", bufs=4, space="DRAM") as bncp:

        # ---- whole-program tiles ----
        kht = [l1.tile([P, TS], DT_B, name=f"kht{i}") for i in range(NHP)]
        qht = [l1.tile([P, TQ], DT_B, name=f"qht{i}") for i in range(NHP)]
        vhp = [l1.tile([P, H, DK], DT_B, name=f"vhp{i}")
               for i in range(NST)]
        oall = [l1.tile([P, TQ], DT_B, name=f"oall{i}") for i in range(NHP)]
        pbt = l1.tile([P, NDCH], DT_F, name="pbt")
        ones = l1.tile([P, 1], DT_B, name="ones")
        nc.sync.dma_start(out=pbt[:, :], in_=pb[:, :])
        nc.vector.memset(ones[:, :], 1.0)

        # ---------- emit helpers ----------
        def emit_kproj_q(hp, j, kin, wkt):
            # quarter j of kht[hp]: token cols j*512:(j+1)*512
            ps = pps.tile([P, 512], DT_F, name=f"kps_{hp}_{j}", tag="pp")
            for d in range(NDCH):
                nc.tensor.matmul(
                    ps[:, :],
                    wkt[d][:, hp * P:(hp + 1) * P],
                    kin[d][:, j * 512:(j + 1) * 512],
                    start=(d == 0), stop=(d == NDCH - 1))
            nc.vector.tensor_copy(kht[hp][:, j * 512:(j + 1) * 512],
                                  ps[:, :])

        def emit_qproj_h(hp, m, qin, wqt):
            ps = pps.tile([P, 512], DT_F, name=f"qps_{hp}_{m}", tag="pp")
            for d in range(NDCH):
                nc.tensor.matmul(
                    ps[:, :],
                    wqt[d][:, hp * P:(hp + 1) * P],
                    qin[d][:, m * 512:(m + 1) * 512],
                    start=(d == 0), stop=(d == NDCH - 1))
            nc.vector.tensor_copy(qht[hp][:, m * 512:(m + 1) * 512],
                                  ps[:, :])

        def emit_vproj(st, vin, wvt):
            # vhp[st][s, h, dk] for all 16 heads; sc-tag psum (prologue)
            ps = scps.tile([P, 1024], DT_F, name=f"vps_{st}", tag="sc")
            for d in range(NDCH):
                for m in range(2):
                    nc.tensor.matmul(
                        ps[:, m * 512:(m + 1) * 512],
                        vin[d][:, st * P:(st + 1) * P],
                        wvt[d][:, m * 512:(m + 1) * 512],
                        start=(d == 0), stop=(d == NDCH - 1))
            nc.vector.tensor_copy(
                vhp[st][:, :, :],
                ps[:, 0:1024].rearrange("p (h d) -> p h d", d=DK))

        def emit_block(hp, weave):
            hA, hB = 2 * hp, 2 * hp + 1
            opq = [ops.tile([P, 512], DT_F, name=f"op_{hp}_{q}", tag="op")
                   for q in range(2)]
            den = dps.tile([P, 512], DT_F, name=f"den_{hp}", tag="den")
            PV_DELAY = 3

            def emit_pvden(kt, exq):
                # PV: A rows 0:64 / B rows 64:128, col-concurrent pairs
                for q in range(2):
                    nc.tensor.matmul(
                        opq[q][0:DK, :], vhp[kt][:, hA, :],
                        exq[q][:, 0:512],
                        start=(kt == 0), stop=(kt == NST - 1))
                    nc.tensor.matmul(
                        opq[q][DK:P, :], vhp[kt][:, hB, :],
                        exq[q][:, 512:1024],
                        start=(kt == 0), stop=(kt == NST - 1))
                # denominators: 4 concurrent M=1 col-tiled matmuls
                # row 32*j with j = 2*q + side
                for j in range(4):
                    q, side = j // 2, j % 2
                    nc.tensor.matmul(
                        den[32 * j:32 * j + 1, :], ones[:, :],
                        exq[q][:, side * 512:(side + 1) * 512],
                        start=(kt == 0), stop=(kt == NST - 1),
                        tile_position=(0, 32 * j))

            pending = []
            for kt in range(NST):
                exq = []
                for q in range(2):
                    sc = scps.tile([P, 1024], DT_F,
                                   name=f"sc_{hp}_{kt}_{q}", tag="sc")
                    nc.tensor.matmul(
                        sc[:, 0:512],
                        kht[hp][0:DK, kt * P:(kt + 1) * P],
                        qht[hp][0:DK, q * 512:(q + 1) * 512],
                        start=True, stop=True)
                    nc.tensor.matmul(
                        sc[:, 512:1024],
                        kht[hp][DK:P, kt * P:(kt + 1) * P],
                        qht[hp][DK:P, q * 512:(q + 1) * 512],
                        start=True, stop=True)
                    ex = expp.tile([P, 1024], DT_B,
                                   name=f"exp_{hp}_{kt}_{q}", tag="exp")
                    nc.scalar.activation(ex[:, :], sc[:, :], AF.Exp,
                                         scale=1.0 / 32.0)
                    exq.append(ex)
                pending.append((kt, exq))
                if len(pending) > PV_DELAY:
                    pkt, pexq = pending.pop(0)
                    emit_pvden(pkt, pexq)
                if kt < len(weave):
                    weave[kt]()
            for pkt, pexq in pending:
                emit_pvden(pkt, pexq)
            # normalization
            sm = smp.tile([P, 512], DT_F, name=f"sm_{hp}", tag="sm")
            for j in range(4):
                nc.vector.reciprocal(sm[32 * j:32 * j + 1, :],
                                     den[32 * j:32 * j + 1, :])
            bcq = [bcp.tile([P, 512], DT_F, name=f"bc_{hp}_{q}", tag="bc")
                   for q in range(2)]
            # den rows 32*j: j=0 A_q0, 1 B_q0, 2 A_q1, 3 B_q1
            for j in range(4):
                q, side = j // 2, j % 2
                bn = bncp.tile([1, 512], DT_F, name=f"bn_{hp}_{j}",
                               tag="bn")
                nc.sync.dma_start(out=bn[:, :],
                                  in_=sm[32 * j:32 * j + 1, :])
                nc.sync.dma_start(
                    out=bcq[q][side * DK:(side + 1) * DK, :],
                    in_=bcast_ap(bn[0:1, :], DK))
            for q in range(2):
                nc.vector.tensor_mul(oall[hp][:, q * 512:(q + 1) * 512],
                                     opq[q][:, :], bcq[q][:, :])

        # ---------- program ----------
        # prologue: V-proj all st, K(0), Q(0)
        with tc.tile_pool(name="v_in", bufs=1) as vip:
            vin = [vip.tile([P, TS], DT_B, name=f"vin{d}")
                   for d in range(NDCH)]
            wvt = [vip.tile([P, H * DK], DT_B, name=f"wvt{d}")
                   for d in range(NDCH)]
            for d in range(NDCH):
                nc.sync.dma_start(out=vin[d][:, :],
                                  in_=vT[d * P:(d + 1) * P, :])
                nc.scalar.dma_start(out=wvt[d][:, :],
                                    in_=wv[d * P:(d + 1) * P, :])
            for st in range(NST):
                emit_vproj(st, vin, wvt)

        with tc.tile_pool(name="kq_in", bufs=1) as kqp:
            kin = [kqp.tile([P, TS], DT_B, name=f"kin{d}")
                   for d in range(NDCH)]
            wkt = [kqp.tile([P, H * DK], DT_B, name=f"wkt{d}")
                   for d in range(NDCH)]
            qin = [kqp.tile([P, TQ], DT_B, name=f"qin{d}")
                   for d in range(NDCH)]
            wqt = [kqp.tile([P, H * DK], DT_B, name=f"wqt{d}")
                   for d in range(NDCH)]
            for d in range(NDCH):
                nc.sync.dma_start(out=kin[d][:, :],
                                  in_=kT[d * P:(d + 1) * P, :])
                nc.scalar.dma_start(out=wkt[d][:, :],
                                    in_=wk[d * P:(d + 1) * P, :])
                nc.gpsimd.dma_start(out=qin[d][:, :],
                                    in_=qT[d * P:(d + 1) * P, :])
                nc.gpsimd.dma_start(out=wqt[d][:, :],
                                    in_=wq[d * P:(d + 1) * P, :])

            for j in range(4):
                emit_kproj_q(0, j, kin, wkt)
            for m in range(2):
                emit_qproj_h(0, m, qin, wqt)

            def weave_for(hp):
                # projection chunks for head pair hp, spread over slots
                if hp >= NHP:
                    return []
                w = []
                for j in range(4):
                    w.append(lambda j=j: emit_kproj_q(hp, j, kin, wkt))
                for m in range(2):
                    w.append(lambda m=m: emit_qproj_h(hp, m, qin, wqt))
                return w

            for hp in range(NHP):
                # spread the ~6 weave thunks across the 16 kt slots
                thunks = weave_for(hp + 1)
                slots = [None] * NST
                for i, t in enumerate(thunks):
                    slots[2 + 2 * i] = t
                weave = [t if t is not None else (lambda: None)
                         for t in slots]
                emit_block(hp, weave)

        # ---- phase C ----
        with tc.tile_pool(name="pw_pool", bufs=1) as pwp, \
             tc.tile_pool(name="yst_pool", bufs=2) as ystp:
            pwsb = [pwp.tile([P, DIM], DT_B, name=f"pwsb{i}")
                    for i in range(NHP)]
            for hp in range(NHP):
                nc.gpsimd.dma_start(out=pwsb[hp][:, :],
                                    in_=pw[hp * P:(hp + 1) * P, :])
            for dt_ in range(NDCH):
                ps = scps.tile([P, 1024], DT_F, name=f"yps_{dt_}",
                               tag="sc")
                for hp in range(NHP):
                    for m in range(2):
                        nc.tensor.matmul(
                            ps[:, m * 512:(m + 1) * 512],
                            pwsb[hp][:, dt_ * P:(dt_ + 1) * P],
                            oall[hp][:, m * 512:(m + 1) * 512],
                            start=(hp == 0), stop=(hp == NHP - 1))
                yst = ystp.tile([P, TQ], DT_F, name=f"yst_{dt_}",
                                tag="yst")
                nc.vector.tensor_scalar_add(yst[:, :], ps[:, 0:1024],
                                            pbt[:, dt_:dt_ + 1])
                nc.sync.dma_start(
                    out=yT[dt_ * P:(dt_ + 1) * P, :], in_=yst[:, :])

    nc.compile()
    return nc


def kernel(q, k, v, w_q, w_k, w_v, proj_w, proj_b):
    global _NC, LAST_RESULT
    import ml_dtypes
    from concourse.bass_utils import run_bass_kernel_spmd

    if _NC is None:
        _NC = _build()

    bf16 = ml_dtypes.bfloat16
    q = np.asarray(q, dtype=np.float32)
    k = np.asarray(k, dtype=np.float32)
    v = np.asarray(v, dtype=np.float32)
    w_q = np.asarray(w_q, dtype=np.float32)
    w_k = np.asarray(w_k, dtype=np.float32)
    w_v = np.asarray(w_v, dtype=np.float32)
    proj_w = np.asarray(proj_w, dtype=np.float32)
    proj_b = np.asarray(proj_b, dtype=np.float32)

    wq2 = np.ascontiguousarray(
        np.transpose(w_q, (1, 0, 2)).reshape(DIM, H * DK)).astype(bf16)
    wk2 = np.ascontiguousarray(
        np.transpose(w_k, (1, 0, 2)).reshape(DIM, H * DK)).astype(bf16)
    wv2 = np.ascontiguousarray(
        np.transpose(w_v, (1, 0, 2)).reshape(DIM, H * DK)).astype(bf16)
    pwT = np.ascontiguousarray(proj_w.T).astype(bf16)
    pb2 = np.ascontiguousarray(proj_b.reshape(NDCH, P).T)

    in_maps = []
    for c in range(N_CORES):
        b, qo = c // 2, c % 2
        if qo == 0:
            kTb = np.ascontiguousarray(k[b].T).astype(bf16)
            vTb = np.ascontiguousarray(v[b].T).astype(bf16)
        in_maps.append({
            "qT": np.ascontiguousarray(
                q[b, qo * TQ:(qo + 1) * TQ, :].T).astype(bf16),
            "kT": kTb,
            "vT": vTb,
            "wq": wq2, "wk": wk2, "wv": wv2,
            "pwT": pwT, "pb": pb2,
        })

    res = run_bass_kernel_spmd(_NC, in_maps, list(range(N_CORES)), trace=TRACE)
    LAST_RESULT = res

    out = np.empty((B, L, DIM), dtype=np.float32)
    for c in range(N_CORES):
        b, qo = c // 2, c % 2
        out[b, qo * TQ:(qo + 1) * TQ, :] = res.results[c]["yT"].T
    return out


# revision 25
# speedup vs baseline: 1.4396x; 1.0018x over previous
"""Multi-head attention kernel for 8 Trainium2 NeuronCores (v3).

Problem: B=4, L=2048, DIM=1024, H=16 heads, d_k=d_v=64.
Sharding: data-parallel over (batch, query-half); full K/V replicated
to both cores of a batch pair at input staging (untimed) -> zero
collectives; each core projects all 2048 kv tokens itself.

Per-core dataflow (bf16 matmuls, fp32 PSUM):
  prologue: V-proj (vhp[st] [128s,16h,64], data-stationary), K(0), Q(0)
  per head-pair hp (blocks):
    per key tile kt:
      scores: 4 matmuls into ONE [128,2048] PSUM tile
        [A|B] x [q0|q1]; A = PE rows 0:64 (tile_position (0,0)),
        B = rows 64:128 ((64,0)) - concurrent row-tiled K=64 pairs.
      ONE 2us ACT(exp, scale=1/32) over all 2048 -> exp tile bf16.
      PV: op_q[128,512] rows 0:64 += V_A.T @ exp_A (tile (0,0)),
          rows 64:128 += V_B.T @ exp_B ((0,64)) - col-concurrent.
      den: 4 concurrent M=1 matmuls (ones stationary) -> den bank
          rows 0/32/64/96 = A_q0/B_q0/A_q1/B_q1.
      weave slot: K/Q projection chunks of hp+1 fill the ACT-gated
          tensor idle time (engine queues are FIFO in emission order,
          so overlap must be emitted interleaved).
    norm: 4x recip -> DRAM bounce -> partition-bcast into bc[0:64]/
          [64:128] -> 2 full [128,512] muls into oall[hp] (PACKED:
          head A rows 0:64, head B rows 64:128; no partition shift
          needed since PV already placed B at 64:128).
  C: yT[dt] = sum_hp pwsb[hp][:,dt-chunk].T @ oall[hp] + bias
     (8 full-K=128 chunks).

PSUM map (8 banks): sc [128,2048] (4) + op bufs=2 (2) + den (1) +
pp [128,512] for woven projections (1).
"""

import numpy as np

P = 128
B, L, DIM, H, DK = 4, 2048, 1024, 16, 64
TQ = 1024      # q tokens per core
TS = 2048      # kv tokens per core (full batch)
NDCH = DIM // P          # 8 contraction chunks
NHP = H // 2             # 8 head pairs
NST = TS // P            # 16 key tiles
N_CORES = 8

_NC = None
TRACE = False
LAST_RESULT = None


def _build():
    import concourse.bass as bass
    from concourse import bacc
    import concourse.mybir as mybir
    import concourse.tile as tile

    DT_B = mybir.dt.bfloat16
    DT_F = mybir.dt.float32
    AF = mybir.ActivationFunctionType

    nc = bacc.Bacc(None, target_bir_lowering=False)
    qT = nc.dram_tensor("qT", [DIM, TQ], DT_B, kind="ExternalInput")
    kT = nc.dram_tensor("kT", [DIM, TS], DT_B, kind="ExternalInput")
    vT = nc.dram_tensor("vT", [DIM, TS], DT_B, kind="ExternalInput")
    wq = nc.dram_tensor("wq", [DIM, H * DK], DT_B, kind="ExternalInput")
    wk = nc.dram_tensor("wk", [DIM, H * DK], DT_B, kind="ExternalInput")
    wv = nc.dram_tensor("wv", [DIM, H * DK], DT_B, kind="ExternalInput")
    pw = nc.dram_tensor("pwT", [H * DK, DIM], DT_B, kind="ExternalInput")
    pb = nc.dram_tensor("pb", [P, NDCH], DT_F, kind="ExternalInput")
    yT = nc.dram_tensor("yT", [DIM, TQ], DT_F, kind="ExternalOutput")

    def bcast_ap(ap, count):
        return bass.AP(tensor=ap.tensor, offset=ap.offset,
                       ap=[[0, count]] + [list(x) for x in ap.ap[1:]])

    with tile.TileContext(nc) as tc, \
         tc.tile_pool(name="l1", bufs=1) as l1, \
         tc.tile_pool(name="exp_pool", bufs=12) as expp, \
         tc.tile_pool(name="scpsum", bufs=2, space="PSUM") as scps, \
         tc.tile_pool(name="opsum", bufs=2, space="PSUM") as ops, \
         tc.tile_pool(name="denpsum", bufs=1, space="PSUM") as dps, \
         tc.tile_pool(name="ppsum", bufs=1, space="PSUM") as pps, \
         tc.tile_pool(name="sums_pool", bufs=1) as smp, \
         tc.tile_pool(name="bc_pool", bufs=2) as bcp, \
         tc.tile_pool(name="bounce", bufs=4, space="DRAM") as bncp:

        # ---- whole-program tiles ----
        kht = [l1.tile([P, TS], DT_B, name=f"kht{i}") for i in range(NHP)]
        qht = [l1.tile([P, TQ], DT_B, name=f"qht{i}") for i in range(NHP)]
        vhp = [l1.tile([P, H, DK], DT_B, name=f"vhp{i}")
               for i in range(NST)]
        oall = [l1.tile([P, TQ], DT_B, name=f"oall{i}") for i in range(NHP)]
        pbt = l1.tile([P, NDCH], DT_F, name="pbt")
        ones = l1.tile([P, 1], DT_B, name="ones")
        nc.sync.dma_start(out=pbt[:, :], in_=pb[:, :])
        nc.vector.memset(ones[:, :], 1.0)

        # ---------- emit helpers ----------
        def emit_kproj_q(hp, j, kin, wkt, pool=None, tag="pp"):
            # quarter j of kht[hp]: token cols j*512:(j+1)*512
            pool = pool or pps
            ps = pool.tile([P, 512], DT_F, name=f"kps_{hp}_{j}", tag=tag)
            for d in range(NDCH):
                nc.tensor.matmul(
                    ps[:, :],
                    wkt[d][:, hp * P:(hp + 1) * P],
                    kin[d][:, j * 512:(j + 1) * 512],
                    start=(d == 0), stop=(d == NDCH - 1))
            nc.vector.tensor_copy(kht[hp][:, j * 512:(j + 1) * 512],
                                  ps[:, :])

        def emit_qproj_h(hp, m, qin, wqt, pool=None, tag="pp"):
            pool = pool or pps
            ps = pool.tile([P, 512], DT_F, name=f"qps_{hp}_{m}", tag=tag)
            for d in range(NDCH):
                nc.tensor.matmul(
                    ps[:, :],
                    wqt[d][:, hp * P:(hp + 1) * P],
                    qin[d][:, m * 512:(m + 1) * 512],
                    start=(d == 0), stop=(d == NDCH - 1))
            nc.vector.tensor_copy(qht[hp][:, m * 512:(m + 1) * 512],
                                  ps[:, :])

        def emit_vproj(st, vin, wvt):
            # vhp[st][s, h, dk] for all 16 heads; sc-tag psum (prologue)
            ps = scps.tile([P, 1024], DT_F, name=f"vps_{st}", tag="sc")
            for d in range(NDCH):
                for m in range(2):
                    nc.tensor.matmul(
                        ps[:, m * 512:(m + 1) * 512],
                        vin[d][:, st * P:(st + 1) * P],
                        wvt[d][:, m * 512:(m + 1) * 512],
                        start=(d == 0), stop=(d == NDCH - 1))
            nc.vector.tensor_copy(
                vhp[st][:, :, :],
                ps[:, 0:1024].rearrange("p (h d) -> p h d", d=DK))

        def emit_block(hp, weave):
            hA, hB = 2 * hp, 2 * hp + 1
            opq = [ops.tile([P, 512], DT_F, name=f"op_{hp}_{q}", tag="op")
                   for q in range(2)]
            den = dps.tile([P, 512], DT_F, name=f"den_{hp}", tag="den")
            PV_DELAY = 5

            def emit_pvden(kt, exq):
                # PV: A rows 0:64 / B rows 64:128, col-concurrent pairs
                for q in range(2):
                    nc.tensor.matmul(
                        opq[q][0:DK, :], vhp[kt][:, hA, :],
                        exq[q][:, 0:512],
                        start=(kt == 0), stop=(kt == NST - 1))
                    nc.tensor.matmul(
                        opq[q][DK:P, :], vhp[kt][:, hB, :],
                        exq[q][:, 512:1024],
                        start=(kt == 0), stop=(kt == NST - 1))
                # denominators: 4 concurrent M=1 col-tiled matmuls
                # row 32*j with j = 2*q + side
                for j in range(4):
                    q, side = j // 2, j % 2
                    nc.tensor.matmul(
                        den[32 * j:32 * j + 1, :], ones[:, :],
                        exq[q][:, side * 512:(side + 1) * 512],
                        start=(kt == 0), stop=(kt == NST - 1),
                        tile_position=(0, 32 * j))

            pending = []
            for kt in range(NST):
                exq = []
                for q in range(2):
                    sc = scps.tile([P, 1024], DT_F,
                                   name=f"sc_{hp}_{kt}_{q}", tag="sc")
                    nc.tensor.matmul(
                        sc[:, 0:512],
                        kht[hp][0:DK, kt * P:(kt + 1) * P],
                        qht[hp][0:DK, q * 512:(q + 1) * 512],
                        start=True, stop=True)
                    nc.tensor.matmul(
                        sc[:, 512:1024],
                        kht[hp][DK:P, kt * P:(kt + 1) * P],
                        qht[hp][DK:P, q * 512:(q + 1) * 512],
                        start=True, stop=True)
                    ex = expp.tile([P, 1024], DT_B,
                                   name=f"exp_{hp}_{kt}_{q}", tag="exp")
                    nc.scalar.activation(ex[:, :], sc[:, :], AF.Exp,
                                         scale=1.0 / 32.0)
                    exq.append(ex)
                pending.append((kt, exq))
                if len(pending) > PV_DELAY:
                    pkt, pexq = pending.pop(0)
                    emit_pvden(pkt, pexq)
                if kt < len(weave):
                    weave[kt]()
            for pkt, pexq in pending:
                emit_pvden(pkt, pexq)
            # normalization
            sm = smp.tile([P, 512], DT_F, name=f"sm_{hp}", tag="sm")
            for j in range(4):
                nc.vector.reciprocal(sm[32 * j:32 * j + 1, :],
                                     den[32 * j:32 * j + 1, :])
            bcq = [bcp.tile([P, 512], DT_F, name=f"bc_{hp}_{q}", tag="bc")
                   for q in range(2)]
            # den rows 32*j: j=0 A_q0, 1 B_q0, 2 A_q1, 3 B_q1
            for j in range(4):
                q, side = j // 2, j % 2
                bn = bncp.tile([1, 512], DT_F, name=f"bn_{hp}_{j}",
                               tag="bn")
                nc.sync.dma_start(out=bn[:, :],
                                  in_=sm[32 * j:32 * j + 1, :])
                nc.sync.dma_start(
                    out=bcq[q][side * DK:(side + 1) * DK, :],
                    in_=bcast_ap(bn[0:1, :], DK))
            for q in range(2):
                nc.vector.tensor_mul(oall[hp][:, q * 512:(q + 1) * 512],
                                     opq[q][:, :], bcq[q][:, :])

        # ---------- program ----------
        # prologue: V-proj all st, K(0), Q(0)
        with tc.tile_pool(name="v_in", bufs=1) as vip:
            vin = [vip.tile([P, TS], DT_B, name=f"vin{d}")
                   for d in range(NDCH)]
            wvt = [vip.tile([P, H * DK], DT_B, name=f"wvt{d}")
                   for d in range(NDCH)]
            for d in range(NDCH):
                nc.sync.dma_start(out=vin[d][:, :],
                                  in_=vT[d * P:(d + 1) * P, :])
                nc.scalar.dma_start(out=wvt[d][:, :],
                                    in_=wv[d * P:(d + 1) * P, :])
            for st in range(NST):
                emit_vproj(st, vin, wvt)

        with tc.tile_pool(name="kq_in", bufs=1) as kqp:
            kin = [kqp.tile([P, TS], DT_B, name=f"kin{d}")
                   for d in range(NDCH)]
            wkt = [kqp.tile([P, H * DK], DT_B, name=f"wkt{d}")
                   for d in range(NDCH)]
            qin = [kqp.tile([P, TQ], DT_B, name=f"qin{d}")
                   for d in range(NDCH)]
            wqt = [kqp.tile([P, H * DK], DT_B, name=f"wqt{d}")
                   for d in range(NDCH)]
            for d in range(NDCH):
                nc.sync.dma_start(out=kin[d][:, :],
                                  in_=kT[d * P:(d + 1) * P, :])
                nc.scalar.dma_start(out=wkt[d][:, :],
                                    in_=wk[d * P:(d + 1) * P, :])
                nc.gpsimd.dma_start(out=qin[d][:, :],
                                    in_=qT[d * P:(d + 1) * P, :])
                nc.gpsimd.dma_start(out=wqt[d][:, :],
                                    in_=wq[d * P:(d + 1) * P, :])

            # prologue K(0)/Q(0): borrow the idle op banks (bufs=2) so
            # quarter-copies double-buffer instead of serializing
            for j in range(4):
                emit_kproj_q(0, j, kin, wkt, pool=ops, tag="op")
            for m in range(2):
                emit_qproj_h(0, m, qin, wqt, pool=ops, tag="op")

            def weave_for(hp):
                # projection chunks for head pair hp, spread over slots
                if hp >= NHP:
                    return []
                w = []
                for j in range(4):
                    w.append(lambda j=j: emit_kproj_q(hp, j, kin, wkt))
                for m in range(2):
                    w.append(lambda m=m: emit_qproj_h(hp, m, qin, wqt))
                return w

            for hp in range(NHP):
                # spread the ~6 weave thunks across the 16 kt slots
                thunks = weave_for(hp + 1)
                slots = [None] * NST
                for i, t in enumerate(thunks):
                    slots[2 + 2 * i] = t
                weave = [t if t is not None else (lambda: None)
                         for t in slots]
                emit_block(hp, weave)

        # ---- phase C ----
        with tc.tile_pool(name="pw_pool", bufs=1) as pwp, \
             tc.tile_pool(name="yst_pool", bufs=2) as ystp:
            pwsb = [pwp.tile([P, DIM], DT_B, name=f"pwsb{i}")
                    for i in range(NHP)]
            for hp in range(NHP):
                nc.gpsimd.dma_start(out=pwsb[hp][:, :],
                                    in_=pw[hp * P:(hp + 1) * P, :])
            for dt_ in range(NDCH):
                ps = scps.tile([P, 1024], DT_F, name=f"yps_{dt_}",
                               tag="sc")
                for hp in range(NHP):
                    for m in range(2):
                        nc.tensor.matmul(
                            ps[:, m * 512:(m + 1) * 512],
                            pwsb[hp][:, dt_ * P:(dt_ + 1) * P],
                            oall[hp][:, m * 512:(m + 1) * 512],
                            start=(hp == 0), stop=(hp == NHP - 1))
                yst = ystp.tile([P, TQ], DT_F, name=f"yst_{dt_}",
                                tag="yst")
                nc.vector.tensor_scalar_add(yst[:, :], ps[:, 0:1024],
                                            pbt[:, dt_:dt_ + 1])
                nc.sync.dma_start(
                    out=yT[dt_ * P:(dt_ + 1) * P, :], in_=yst[:, :])

    nc.compile()
    return nc


def kernel(q, k, v, w_q, w_k, w_v, proj_w, proj_b):
    global _NC, LAST_RESULT
    import ml_dtypes
    from concourse.bass_utils import run_bass_kernel_spmd

    if _NC is None:
        _NC = _build()

    bf16 = ml_dtypes.bfloat16
    q = np.asarray(q, dtype=np.float32)
    k = np.asarray(k, dtype=np.float32)
    v = np.asarray(v, dtype=np.float32)
    w_q = np.asarray(w_q, dtype=np.float32)
    w_k = np.asarray(w_k, dtype=np.float32)
    w_v = np.asarray(w_v, dtype=np.float32)
    proj_w = np.asarray(proj_w, dtype=np.float32)
    proj_b = np.asarray(proj_b, dtype=np.float32)

    wq2 = np.ascontiguousarray(
        np.transpose(w_q, (1, 0, 2)).reshape(DIM, H * DK)).astype(bf16)
    wk2 = np.ascontiguousarray(
        np.transpose(w_k, (1, 0, 2)).reshape(DIM, H * DK)).astype(bf16)
    wv2 = np.ascontiguousarray(
        np.transpose(w_v, (1, 0, 2)).reshape(DIM, H * DK)).astype(bf16)
    pwT = np.ascontiguousarray(proj_w.T).astype(bf16)
    pb2 = np.ascontiguousarray(proj_b.reshape(NDCH, P).T)

    in_maps = []
    for c in range(N_CORES):
        b, qo = c // 2, c % 2
        if qo == 0:
            kTb = np.ascontiguousarray(k[b].T).astype(bf16)
            vTb = np.ascontiguousarray(v[b].T).astype(bf16)
        in_maps.append({
            "qT": np.ascontiguousarray(
                q[b, qo * TQ:(qo + 1) * TQ, :].T).astype(bf16),
            "kT": kTb,
            "vT": vTb,
            "wq": wq2, "wk": wk2, "wv": wv2,
            "pwT": pwT, "pb": pb2,
        })

    res = run_bass_kernel_spmd(_NC, in_maps, list(range(N_CORES)), trace=TRACE)
    LAST_RESULT = res

    out = np.empty((B, L, DIM), dtype=np.float32)
    for c in range(N_CORES):
        b, qo = c // 2, c % 2
        out[b, qo * TQ:(qo + 1) * TQ, :] = res.results[c]["yT"].T
    return out


# revision 29
# speedup vs baseline: 1.4719x; 1.0224x over previous
"""Multi-head attention kernel for 8 Trainium2 NeuronCores (v3).

Problem: B=4, L=2048, DIM=1024, H=16 heads, d_k=d_v=64.
Sharding: data-parallel over (batch, query-half); full K/V replicated
to both cores of a batch pair at input staging (untimed) -> zero
collectives; each core projects all 2048 kv tokens itself.

Per-core dataflow (bf16 matmuls, fp32 PSUM):
  prologue: V-proj (vhp[st] [128s,16h,64], data-stationary), K(0), Q(0)
  per head-pair hp (blocks):
    per key tile kt:
      scores: 4 matmuls into ONE [128,2048] PSUM tile
        [A|B] x [q0|q1]; A = PE rows 0:64 (tile_position (0,0)),
        B = rows 64:128 ((64,0)) - concurrent row-tiled K=64 pairs.
      ONE 2us ACT(exp, scale=1/32) over all 2048 -> exp tile bf16.
      PV: op_q[128,512] rows 0:64 += V_A.T @ exp_A (tile (0,0)),
          rows 64:128 += V_B.T @ exp_B ((0,64)) - col-concurrent.
      den: 4 concurrent M=1 matmuls (ones stationary) -> den bank
          rows 0/32/64/96 = A_q0/B_q0/A_q1/B_q1.
      weave slot: K/Q projection chunks of hp+1 fill the ACT-gated
          tensor idle time (engine queues are FIFO in emission order,
          so overlap must be emitted interleaved).
    norm: 4x recip -> DRAM bounce -> partition-bcast into bc[0:64]/
          [64:128] -> 2 full [128,512] muls into oall[hp] (PACKED:
          head A rows 0:64, head B rows 64:128; no partition shift
          needed since PV already placed B at 64:128).
  C: yT[dt] = sum_hp pwsb[hp][:,dt-chunk].T @ oall[hp] + bias
     (8 full-K=128 chunks).

PSUM map (8 banks): sc [128,2048] (4) + op bufs=2 (2) + den (1) +
pp [128,512] for woven projections (1).
"""

import numpy as np

P = 128
B, L, DIM, H, DK = 4, 2048, 1024, 16, 64
TQ = 1024      # q tokens per core
TS = 2048      # kv tokens per core (full batch)
NDCH = DIM // P          # 8 contraction chunks
NHP = H // 2             # 8 head pairs
NST = TS // P            # 16 key tiles
N_CORES = 8

_NC = None
TRACE = False
LAST_RESULT = None


def _build():
    import concourse.bass as bass
    from concourse import bacc
    import concourse.mybir as mybir
    import concourse.tile as tile

    DT_B = mybir.dt.bfloat16
    DT_F = mybir.dt.float32
    AF = mybir.ActivationFunctionType

    nc = bacc.Bacc(None, target_bir_lowering=False)
    qT = nc.dram_tensor("qT", [DIM, TQ], DT_B, kind="ExternalInput")
    kT = nc.dram_tensor("kT", [DIM, TS], DT_B, kind="ExternalInput")
    vT = nc.dram_tensor("vT", [DIM, TS], DT_B, kind="ExternalInput")
    wq = nc.dram_tensor("wq", [DIM, H * DK], DT_B, kind="ExternalInput")
    wk = nc.dram_tensor("wk", [DIM, H * DK], DT_B, kind="ExternalInput")
    wv = nc.dram_tensor("wv", [DIM, H * DK], DT_B, kind="ExternalInput")
    pw = nc.dram_tensor("pwT", [H * DK, DIM], DT_B, kind="ExternalInput")
    pb = nc.dram_tensor("pb", [P, NDCH], DT_F, kind="ExternalInput")
    yT = nc.dram_tensor("yT", [DIM, TQ], DT_F, kind="ExternalOutput")

    def bcast_ap(ap, count):
        return bass.AP(tensor=ap.tensor, offset=ap.offset,
                       ap=[[0, count]] + [list(x) for x in ap.ap[1:]])

    with tile.TileContext(nc) as tc, \
         tc.tile_pool(name="l1", bufs=1) as l1, \
         tc.tile_pool(name="exp_pool", bufs=12) as expp, \
         tc.tile_pool(name="scpsum", bufs=2, space="PSUM") as scps, \
         tc.tile_pool(name="opsum", bufs=2, space="PSUM") as ops, \
         tc.tile_pool(name="denpsum", bufs=1, space="PSUM") as dps, \
         tc.tile_pool(name="ppsum", bufs=1, space="PSUM") as pps, \
         tc.tile_pool(name="sums_pool", bufs=1) as smp, \
         tc.tile_pool(name="bc_pool", bufs=2) as bcp, \
         tc.tile_pool(name="bounce", bufs=4, space="DRAM") as bncp:

        # ---- whole-program tiles ----
        kht = [l1.tile([P, TS], DT_B, name=f"kht{i}") for i in range(NHP)]
        qht = [l1.tile([P, TQ], DT_B, name=f"qht{i}") for i in range(NHP)]
        vhp = [l1.tile([P, H, DK], DT_B, name=f"vhp{i}")
               for i in range(NST)]
        oall = [l1.tile([P, TQ], DT_B, name=f"oall{i}") for i in range(NHP)]
        pbt = l1.tile([P, NDCH], DT_F, name="pbt")
        ones = l1.tile([P, 1], DT_B, name="ones")
        nc.sync.dma_start(out=pbt[:, :], in_=pb[:, :])
        nc.vector.memset(ones[:, :], 1.0)

        # ---------- emit helpers ----------
        def emit_kproj_q(hp, j, kin, wkt, pool=None, tag="pp"):
            # quarter j of kht[hp]: token cols j*512:(j+1)*512
            pool = pool or pps
            ps = pool.tile([P, 512], DT_F, name=f"kps_{hp}_{j}", tag=tag)
            for d in range(NDCH):
                nc.tensor.matmul(
                    ps[:, :],
                    wkt[d][:, hp * P:(hp + 1) * P],
                    kin[d][:, j * 512:(j + 1) * 512],
                    start=(d == 0), stop=(d == NDCH - 1))
            nc.vector.tensor_copy(kht[hp][:, j * 512:(j + 1) * 512],
                                  ps[:, :])

        def emit_qproj_h(hp, m, qin, wqt, pool=None, tag="pp"):
            pool = pool or pps
            ps = pool.tile([P, 512], DT_F, name=f"qps_{hp}_{m}", tag=tag)
            for d in range(NDCH):
                nc.tensor.matmul(
                    ps[:, :],
                    wqt[d][:, hp * P:(hp + 1) * P],
                    qin[d][:, m * 512:(m + 1) * 512],
                    start=(d == 0), stop=(d == NDCH - 1))
            nc.vector.tensor_copy(qht[hp][:, m * 512:(m + 1) * 512],
                                  ps[:, :])

        def emit_vproj(st, vin, wvt):
            # vhp[st][s, h, dk] for all 16 heads; sc-tag psum (prologue)
            ps = scps.tile([P, 1024], DT_F, name=f"vps_{st}", tag="sc")
            for d in range(NDCH):
                for m in range(2):
                    nc.tensor.matmul(
                        ps[:, m * 512:(m + 1) * 512],
                        vin[d][:, st * P:(st + 1) * P],
                        wvt[d][:, m * 512:(m + 1) * 512],
                        start=(d == 0), stop=(d == NDCH - 1))
            nc.vector.tensor_copy(
                vhp[st][:, :, :],
                ps[:, 0:1024].rearrange("p (h d) -> p h d", d=DK))

        def emit_block(hp, weave):
            hA, hB = 2 * hp, 2 * hp + 1
            opq = [ops.tile([P, 512], DT_F, name=f"op_{hp}_{q}", tag="op")
                   for q in range(2)]
            den = dps.tile([P, 512], DT_F, name=f"den_{hp}", tag="den")
            PV_DELAY = 3

            def emit_pvden(kt, exq):
                # PV: A rows 0:64 / B rows 64:128, col-concurrent pairs
                for q in range(2):
                    nc.tensor.matmul(
                        opq[q][0:DK, :], vhp[kt][:, hA, :],
                        exq[q][:, 0:512],
                        start=(kt == 0), stop=(kt == NST - 1))
                    nc.tensor.matmul(
                        opq[q][DK:P, :], vhp[kt][:, hB, :],
                        exq[q][:, 512:1024],
                        start=(kt == 0), stop=(kt == NST - 1))
                # denominators: 4 concurrent M=1 col-tiled matmuls
                # row 32*j with j = 2*q + side
                for j in range(4):
                    q, side = j // 2, j % 2
                    nc.tensor.matmul(
                        den[32 * j:32 * j + 1, :], ones[:, :],
                        exq[q][:, side * 512:(side + 1) * 512],
                        start=(kt == 0), stop=(kt == NST - 1),
                        tile_position=(0, 32 * j))

            # run leading weave thunks first: they fill the PE while the
            # previous block's ACT tail drains the sc buffers
            lead = min(3, len(weave_real := [w for w in weave
                                             if w is not None]))
            for w in weave_real[:lead]:
                w()
            rest = weave_real[lead:]
            pending = []
            for kt in range(NST):
                exq = []
                for q in range(2):
                    sc = scps.tile([P, 1024], DT_F,
                                   name=f"sc_{hp}_{kt}_{q}", tag="sc")
                    nc.tensor.matmul(
                        sc[:, 0:512],
                        kht[hp][0:DK, kt * P:(kt + 1) * P],
                        qht[hp][0:DK, q * 512:(q + 1) * 512],
                        start=True, stop=True)
                    nc.tensor.matmul(
                        sc[:, 512:1024],
                        kht[hp][DK:P, kt * P:(kt + 1) * P],
                        qht[hp][DK:P, q * 512:(q + 1) * 512],
                        start=True, stop=True)
                    ex = expp.tile([P, 1024], DT_B,
                                   name=f"exp_{hp}_{kt}_{q}", tag="exp")
                    nc.scalar.activation(ex[:, :], sc[:, :], AF.Exp,
                                         scale=1.0 / 32.0)
                    exq.append(ex)
                pending.append((kt, exq))
                if len(pending) > PV_DELAY:
                    pkt, pexq = pending.pop(0)
                    emit_pvden(pkt, pexq)
                if kt % 3 == 2 and rest:
                    rest.pop(0)()
            for w in rest:
                w()
            for pkt, pexq in pending:
                emit_pvden(pkt, pexq)
            # normalization
            sm = smp.tile([P, 512], DT_F, name=f"sm_{hp}", tag="sm")
            for j in range(4):
                nc.vector.reciprocal(sm[32 * j:32 * j + 1, :],
                                     den[32 * j:32 * j + 1, :])
            bcq = [bcp.tile([P, 512], DT_F, name=f"bc_{hp}_{q}", tag="bc")
                   for q in range(2)]
            # den rows 32*j: j=0 A_q0, 1 B_q0, 2 A_q1, 3 B_q1
            for j in range(4):
                q, side = j // 2, j % 2
                bn = bncp.tile([1, 512], DT_F, name=f"bn_{hp}_{j}",
                               tag="bn")
                nc.sync.dma_start(out=bn[:, :],
                                  in_=sm[32 * j:32 * j + 1, :])
                nc.sync.dma_start(
                    out=bcq[q][side * DK:(side + 1) * DK, :],
                    in_=bcast_ap(bn[0:1, :], DK))
            for q in range(2):
                nc.vector.tensor_mul(oall[hp][:, q * 512:(q + 1) * 512],
                                     opq[q][:, :], bcq[q][:, :])

        # ---------- program ----------
        # prologue: V-proj all st, K(0), Q(0)
        with tc.tile_pool(name="v_in", bufs=1) as vip:
            vin = [vip.tile([P, TS], DT_B, name=f"vin{d}")
                   for d in range(NDCH)]
            wvt = [vip.tile([P, H * DK], DT_B, name=f"wvt{d}")
                   for d in range(NDCH)]
            for d in range(NDCH):
                nc.sync.dma_start(out=vin[d][:, :],
                                  in_=vT[d * P:(d + 1) * P, :])
                nc.scalar.dma_start(out=wvt[d][:, :],
                                    in_=wv[d * P:(d + 1) * P, :])
            for st in range(NST):
                emit_vproj(st, vin, wvt)

        with tc.tile_pool(name="kq_in", bufs=1) as kqp:
            kin = [kqp.tile([P, TS], DT_B, name=f"kin{d}")
                   for d in range(NDCH)]
            wkt = [kqp.tile([P, H * DK], DT_B, name=f"wkt{d}")
                   for d in range(NDCH)]
            qin = [kqp.tile([P, TQ], DT_B, name=f"qin{d}")
                   for d in range(NDCH)]
            wqt = [kqp.tile([P, H * DK], DT_B, name=f"wqt{d}")
                   for d in range(NDCH)]
            for d in range(NDCH):
                nc.sync.dma_start(out=kin[d][:, :],
                                  in_=kT[d * P:(d + 1) * P, :])
                nc.scalar.dma_start(out=wkt[d][:, :],
                                    in_=wk[d * P:(d + 1) * P, :])
                nc.gpsimd.dma_start(out=qin[d][:, :],
                                    in_=qT[d * P:(d + 1) * P, :])
                nc.gpsimd.dma_start(out=wqt[d][:, :],
                                    in_=wq[d * P:(d + 1) * P, :])

            # prologue K(0)/Q(0): borrow the idle op banks (bufs=2) so
            # quarter-copies double-buffer instead of serializing
            for j in range(4):
                emit_kproj_q(0, j, kin, wkt, pool=ops, tag="op")
            for m in range(2):
                emit_qproj_h(0, m, qin, wqt, pool=ops, tag="op")

            def weave_for(hp):
                # projection chunks for head pair hp, spread over slots
                if hp >= NHP:
                    return []
                w = []
                for j in range(4):
                    w.append(lambda j=j: emit_kproj_q(hp, j, kin, wkt))
                for m in range(2):
                    w.append(lambda m=m: emit_qproj_h(hp, m, qin, wqt))
                return w

            for hp in range(NHP):
                emit_block(hp, weave_for(hp + 1))

        # ---- phase C ----
        with tc.tile_pool(name="pw_pool", bufs=1) as pwp, \
             tc.tile_pool(name="yst_pool", bufs=2) as ystp:
            pwsb = [pwp.tile([P, DIM], DT_B, name=f"pwsb{i}")
                    for i in range(NHP)]
            for hp in range(NHP):
                nc.gpsimd.dma_start(out=pwsb[hp][:, :],
                                    in_=pw[hp * P:(hp + 1) * P, :])
            for dt_ in range(NDCH):
                ps = scps.tile([P, 1024], DT_F, name=f"yps_{dt_}",
                               tag="sc")
                for hp in range(NHP):
                    for m in range(2):
                        nc.tensor.matmul(
                            ps[:, m * 512:(m + 1) * 512],
                            pwsb[hp][:, dt_ * P:(dt_ + 1) * P],
                            oall[hp][:, m * 512:(m + 1) * 512],
                            start=(hp == 0), stop=(hp == NHP - 1))
                yst = ystp.tile([P, TQ], DT_F, name=f"yst_{dt_}",
                                tag="yst")
                nc.vector.tensor_scalar_add(yst[:, :], ps[:, 0:1024],
                                            pbt[:, dt_:dt_ + 1])
                nc.sync.dma_start(
                    out=yT[dt_ * P:(dt_ + 1) * P, :], in_=yst[:, :])

    nc.compile()
    return nc


def kernel(q, k, v, w_q, w_k, w_v, proj_w, proj_b):
    global _NC, LAST_RESULT
    import ml_dtypes
    from concourse.bass_utils import run_bass_kernel_spmd

    if _NC is None:
        _NC = _build()

    bf16 = ml_dtypes.bfloat16
    q = np.asarray(q, dtype=np.float32)
    k = np.asarray(k, dtype=np.float32)
    v = np.asarray(v, dtype=np.float32)
    w_q = np.asarray(w_q, dtype=np.float32)
    w_k = np.asarray(w_k, dtype=np.float32)
    w_v = np.asarray(w_v, dtype=np.float32)
    proj_w = np.asarray(proj_w, dtype=np.float32)
    proj_b = np.asarray(proj_b, dtype=np.float32)

    wq2 = np.ascontiguousarray(
        np.transpose(w_q, (1, 0, 2)).reshape(DIM, H * DK)).astype(bf16)
    wk2 = np.ascontiguousarray(
        np.transpose(w_k, (1, 0, 2)).reshape(DIM, H * DK)).astype(bf16)
    wv2 = np.ascontiguousarray(
        np.transpose(w_v, (1, 0, 2)).reshape(DIM, H * DK)).astype(bf16)
    pwT = np.ascontiguousarray(proj_w.T).astype(bf16)
    pb2 = np.ascontiguousarray(proj_b.reshape(NDCH, P).T)

    in_maps = []
    for c in range(N_CORES):
        b, qo = c // 2, c % 2
        if qo == 0:
            kTb = np.ascontiguousarray(k[b].T).astype(bf16)
            vTb = np.ascontiguousarray(v[b].T).astype(bf16)
        in_maps.append({
            "qT": np.ascontiguousarray(
                q[b, qo * TQ:(qo + 1) * TQ, :].T).astype(bf16),
            "kT": kTb,
            "vT": vTb,
            "wq": wq2, "wk": wk2, "wv": wv2,
            "pwT": pwT, "pb": pb2,
        })

    res = run_bass_kernel_spmd(_NC, in_maps, list(range(N_CORES)), trace=TRACE)
    LAST_RESULT = res

    out = np.empty((B, L, DIM), dtype=np.float32)
    for c in range(N_CORES):
        b, qo = c // 2, c % 2
        out[b, qo * TQ:(qo + 1) * TQ, :] = res.results[c]["yT"].T
    return out
